# revision 6
# baseline (speedup 1.0000x reference)
"""AWGN channel kernel for Trainium2, 8-core data-parallel SPMD.

Math (from the nn.Module):
    signl_pwr = sum(x^2) / numel(x)            # global over the full tensor
    stddev    = sqrt(signl_pwr / snr)          # snr = 10^(10dB/10) = 10
    out       = complex(x + stddev*noise_r, stddev*noise_i)
    h         = ones_like(x)                   # constant, produced host-side

Sharding: batch dim (64) split 8 ways. Per core (8 MB x, 8+8 MB noise):
  phase 1: stream the x shard into SBUF (kept resident); fused
           square+row-accumulate per tile, alternating ScalarE (ACT
           Square+accum_out) and VectorE (scalar_tensor_tensor x*x with
           accum_out) so the square chain halves; PE matmul against a
           ones matrix sums the 128 per-partition partials and
           broadcasts the local sum to all partitions.
  AllReduce of a [128] f32 vector (each lane = local sum) across the 8
           cores -> every lane on every core = global sum.  A dummy
           AllReduce is issued at kernel start so the real one doesn't
           pay the ncfw cold-start (~30 us) on its critical path; the
           same warmup chain preloads ACT's Sqrt table.
  phase 2: s = sqrt(global_sum/(numel*snr)) via ACT Sqrt + one Newton
           step (ACT's LUT alone is only ~1e-4 accurate); stream
           noise_r/noise_i tiles (deep bufs so they prefetch during the
           collective); DVE writes the real part (x + s*nr) to even f32
           slots of an interleaved output tile, ACT writes the imag
           part (s*ni) to odd slots; DMA out.

The interleaved f32 output IS complex64 memory layout, so the host just
.view(np.complex64)s it — no host compute on the hot data.

NB: InstTensorTensorReduce (vector.tensor_tensor_reduce) wedges this
runtime's devices (verified in featurecheck.py) — do not use it.
"""

import sys

import numpy as np

try:
    import concourse.bass as bass  # noqa: F401
except ImportError:  # pragma: no cover - fresh grading dir without PYTHONPATH
    for p in ("/opt/trn_rl_repo", "/root/.axon_site/_ro/trn_rl_repo"):
        if p not in sys.path:
            sys.path.insert(0, p)
    import concourse.bass as bass  # noqa: F401

import concourse.bacc as bacc
import concourse.mybir as mybir
import concourse.tile as tile
from concourse.bass_utils import run_bass_kernel_spmd

N_CORES = 8
FULL_BATCH = 64
SHAPE_TAIL = (16, 128, 128)
PER_CORE_BATCH = FULL_BATCH // N_CORES
ELEMS = PER_CORE_BATCH * 16 * 128 * 128  # 2_097_152 per core
P = 128
FREE = ELEMS // P  # 16384
NT = 8  # tiles per shard
TF = FREE // NT  # 2048 f32 per partition per tile

K_TOTAL = FULL_BATCH * 16 * 128 * 128  # 16_777_216
SNR = 10.0 ** (10.0 / 10.0)
SCALE_C = 1.0 / (K_TOTAL * SNR)  # s = sqrt(global_sum * SCALE_C)

F32 = mybir.dt.float32


def build_nc(reps: int = 1):
    """Build + compile the 8-core SPMD Bass module.

    reps > 1 repeats the whole body (used for steady-state timing by
    differencing); the graded kernel uses reps=1.
    """
    nc = bacc.Bacc(
        "TRN2", target_bir_lowering=False, debug=False, num_devices=N_CORES
    )
    x_d = nc.dram_tensor("x", [P, FREE], F32, kind="ExternalInput").ap()
    nr_d = nc.dram_tensor("nr", [P, FREE], F32, kind="ExternalInput").ap()
    ni_d = nc.dram_tensor("ni", [P, FREE], F32, kind="ExternalInput").ap()
    out_d = nc.dram_tensor("out", [P, 2 * FREE], F32, kind="ExternalOutput").ap()

    with tile.TileContext(nc) as tc:
        with (
            tc.tile_pool(name="xres", bufs=NT) as xpool,
            tc.tile_pool(name="nrp", bufs=6) as nrpool,
            tc.tile_pool(name="nip", bufs=5) as nipool,
            tc.tile_pool(name="outp", bufs=2) as opool,
            tc.tile_pool(name="smalls", bufs=2) as small,
            tc.tile_pool(name="consts", bufs=1) as consts,
            tc.tile_pool(name="psum", bufs=2, space="PSUM") as psum,
            tc.tile_pool(name="dram", bufs=2, space="DRAM") as dram,
        ):
            ones_t = consts.tile([P, P], F32)
            nc.vector.memset(ones_t[:], 1.0)

            # Warmup chain: preload ACT's Sqrt table AND fire a dummy
            # AllReduce so ncfw's cold-start overlaps phase 1 instead of
            # sitting on the real collective's critical path.
            w_sq = small.tile([P, 1], F32, tag="w_sq")
            nc.scalar.activation(
                w_sq[:], ones_t[:, 0:1], mybir.ActivationFunctionType.Sqrt
            )
            cc_w_in = dram.tile([P], F32, tag="cc_w_in")
            cc_w_out = dram.tile([P], F32, tag="cc_w_out", addr_space="Shared")
            nc.sync.dma_start(out=cc_w_in[:], in_=w_sq[:, 0])
            nc.gpsimd.collective_compute(
                "AllReduce",
                mybir.AluOpType.add,
                ins=[cc_w_in[:]],
                outs=[cc_w_out[:]],
                replica_groups=[list(range(N_CORES))],
            )

            for _ in range(reps):
                # ---- phase 1: local sum of squares ----
                acc = small.tile([P, NT], F32, tag="acc")
                xts = []
                for t in range(NT):
                    xt = xpool.tile([P, TF], F32, tag="x")
                    nc.sync.dma_start(out=xt[:], in_=x_d[:, t * TF : (t + 1) * TF])
                    xts.append(xt)
                    sq = opool.tile([P, TF], F32, tag="out")
                    if t % 2 == 0:
                        nc.scalar.activation(
                            sq[:],
                            xt[:],
                            mybir.ActivationFunctionType.Square,
                            accum_out=acc[:, t : t + 1],
                        )
                    else:
                        nc.vector.scalar_tensor_tensor(
                            out=sq[:],
                            in0=xt[:],
                            scalar=1.0,
                            in1=xt[:],
                            op0=mybir.AluOpType.mult,
                            op1=mybir.AluOpType.mult,
                            accum_out=acc[:, t : t + 1],
                        )
                part = small.tile([P, 1], F32, tag="part")
                nc.vector.reduce_sum(part[:], acc[:], axis=mybir.AxisListType.X)
                # sum over partitions + broadcast: ones[128,128]^T @ part
                ps = psum.tile([P, 1], F32, tag="ps")
                nc.tensor.matmul(ps[:], ones_t[:], part[:], start=True, stop=True)
                loc = small.tile([P, 1], F32, tag="loc")
                nc.scalar.copy(loc[:], ps[:])

                # ---- all-reduce the (replicated) local sum across cores ----
                cc_in = dram.tile([P], F32, tag="cc_in")
                cc_out = dram.tile([P], F32, tag="cc_out", addr_space="Shared")
                nc.sync.dma_start(out=cc_in[:], in_=loc[:, 0])
                nc.gpsimd.collective_compute(
                    "AllReduce",
                    mybir.AluOpType.add,
                    ins=[cc_in[:]],
                    outs=[cc_out[:]],
                    replica_groups=[list(range(N_CORES))],
                )
                g = small.tile([P, 1], F32, tag="g")
                nc.sync.dma_start(out=g[:, 0], in_=cc_out[:])

                # s = sqrt(global_sum / (numel * snr)) with one Newton step
                t_pw = small.tile([P, 1], F32, tag="t_pw")
                nc.scalar.activation(
                    t_pw[:], g[:], mybir.ActivationFunctionType.Copy, scale=SCALE_C
                )
                y0 = small.tile([P, 1], F32, tag="y0")
                nc.scalar.activation(
                    y0[:], t_pw[:], mybir.ActivationFunctionType.Sqrt
                )
                r0 = small.tile([P, 1], F32, tag="r0")
                nc.vector.reciprocal(r0[:], y0[:])
                q0 = small.tile([P, 1], F32, tag="q0")
                nc.vector.tensor_mul(q0[:], t_pw[:], r0[:])
                u0 = small.tile([P, 1], F32, tag="u0")
                nc.vector.tensor_add(u0[:], y0[:], q0[:])
                s = small.tile([P, 1], F32, tag="s")
                nc.vector.tensor_scalar_mul(s[:], u0[:], 0.5)

                # ---- phase 2: out_c = (x + s*nr) + i*(s*ni), interleaved ----
                for t in range(NT):
                    nrt = nrpool.tile([P, TF], F32, tag="nr")
                    nit = nipool.tile([P, TF], F32, tag="ni")
                    nc.sync.dma_start(out=nrt[:], in_=nr_d[:, t * TF : (t + 1) * TF])
                    nc.sync.dma_start(out=nit[:], in_=ni_d[:, t * TF : (t + 1) * TF])
                    ot = opool.tile([P, 2 * TF], F32, tag="out")
                    # real part -> even f32 slots
                    nc.vector.scalar_tensor_tensor(
                        out=ot[:, 0 : 2 * TF : 2],
                        in0=nrt[:],
                        scalar=s[:],
                        in1=xts[t][:],
                        op0=mybir.AluOpType.mult,
                        op1=mybir.AluOpType.add,
                    )
                    # imag part -> odd f32 slots
                    nc.scalar.activation(
                        ot[:, 1 : 2 * TF : 2],
                        nit[:],
                        mybir.ActivationFunctionType.Copy,
                        scale=s[:],
                    )
                    nc.sync.dma_start(
                        out=out_d[:, t * 2 * TF : (t + 1) * 2 * TF], in_=ot[:]
                    )
    nc.compile()
    return nc


_NC_CACHE: dict = {}


def get_nc(reps: int = 1):
    if reps not in _NC_CACHE:
        _NC_CACHE[reps] = build_nc(reps)
    return _NC_CACHE[reps]


def _shard(arr: np.ndarray, core: int) -> np.ndarray:
    lo = core * PER_CORE_BATCH
    return arr[lo : lo + PER_CORE_BATCH].reshape(P, FREE)


def kernel(channal_input, P=None, noise_r=None, noise_i=None):  # noqa: N803
    x = np.asarray(channal_input, dtype=np.float32)
    nr = np.asarray(noise_r, dtype=np.float32)
    ni = np.asarray(noise_i, dtype=np.float32)
    assert x.shape == (FULL_BATCH, *SHAPE_TAIL), x.shape

    nc = get_nc(1)
    in_maps = [
        {"x": _shard(x, c), "nr": _shard(nr, c), "ni": _shard(ni, c)}
        for c in range(N_CORES)
    ]
    res = run_bass_kernel_spmd(nc, in_maps, list(range(N_CORES)))

    out = np.empty((FULL_BATCH, *SHAPE_TAIL), dtype=np.complex64)
    for c in range(N_CORES):
        lo = c * PER_CORE_BATCH
        out[lo : lo + PER_CORE_BATCH] = (
            res.results[c]["out"]
            .reshape(-1)
            .view(np.complex64)
            .reshape(PER_CORE_BATCH, *SHAPE_TAIL)
        )
    h = np.ones((FULL_BATCH, *SHAPE_TAIL), dtype=np.float32)
    return out, h
